# revision 28
# baseline (speedup 1.0000x reference)
"""B-spline basis kernel for Trainium2 (8 NeuronCores, data-parallel over rows).

Computes B[n, j] = cubic B-spline basis (uniform knots on [-1, 1], 64 basis
functions, degree 3) for x[n], n = 0..2097151.  Output [N, 64] f32.

Math: with uniform non-repeated knots every basis function is a shifted copy
of the cardinal cubic B-spline b3 (support (0, 4)):
    B[n, j] = b3(s_n - j),   s_n = (x_n + 1) / h,  h = 2/67

Implementation: b3 is itself a 4-piece cubic polynomial — exactly what the
ScalarEngine's activation hardware evaluates (piecewise-cubic bucket tables).
We install b3 as a custom activation table (overriding the `gelu` slot via
--act-root-json) so the whole computation is:
    t[p, f, j] = x[p, f] * 33.5 - (j - 33.5)        one fused DVE op
    out        = b3(t)                               one ACT op
    DMA out                                          (memory-bound)
"""
import sys, os, json, shutil, struct, hashlib, tempfile

sys.path.insert(0, "/opt/trn_rl_repo")
from contextlib import ExitStack

import numpy as np

N = 2097152
NUM_BASIS = 64
INV_H = 33.5                           # 1/h = 67/2, exact in fp32
N_CORES = 8
ROWS_PER_CORE = N // N_CORES           # 262144

RPP = 64                               # rows per partition per tile
TILE_F = RPP * NUM_BASIS               # 4096 f32 per partition (16 KiB)
TILE_ROWS = 128 * RPP                  # 8192 rows per tile
N_TILES = ROWS_PER_CORE // TILE_ROWS   # 32

_CACHE = {}

# ---------------------------------------------------------------------------
# custom ACT table: b3 on the gelu slot
# ---------------------------------------------------------------------------
_PIECES = {
    0: ([0.0, 0.0, 0.0, 1.0 / 6.0], 0.0),
    1: ([1.0 / 6.0, 0.5, 0.5, -0.5], 1.0),
    2: ([2.0 / 3.0, 0.0, -1.0, 0.5], 2.0),
    3: ([1.0 / 6.0, -0.5, 0.5, -1.0 / 6.0], 3.0),
}
_ZERO = ([0.0, 0.0, 0.0, 0.0], 0.0)


def _default_act_root():
    from neuronxcc.driver.Job import Job
    from neuronxcc.driver.jobs.support.FindActInfo import findActInfoFile
    return os.path.dirname(findActInfoFile(Job.getPackageDir(), "gen3"))


def _build_act_root():
    src = _default_act_root()
    dst = os.path.join(tempfile.gettempdir(), f"b3_act_root_{os.getpid()}")
    os.makedirs(dst, exist_ok=True)
    for f in os.listdir(src):
        sp = os.path.join(src, f)
        if os.path.isfile(sp):
            dp = os.path.join(dst, f)
            if os.path.exists(dp):
                os.chmod(dp, 0o644)
                os.remove(dp)
            shutil.copy(sp, dp)
            os.chmod(dp, 0o644)

    prof_path = os.path.join(dst, "gelu_and_others.json")
    prof = json.load(open(prof_path))
    bkt = bytearray(open(os.path.join(dst, "gelu_and_others_bkt.bin"), "rb").read())

    e2b = prof["func_exp_to_bkt_start_idx"]["gelu"]
    exps = sorted(int(e) for e in e2b)
    neg_starts = {e: e2b[str(e)][0] for e in exps}
    pos_starts = {e: e2b[str(e)][1] for e in exps if len(e2b[str(e)]) > 1}

    meta = next(m for m in prof["profile_meta_data"] if m["func_id"] == 23)

    bkts = {}

    def fill(start, count, fn):
        for b in range(count):
            bkts[start + b] = fn(b, count)

    neg_sorted = [neg_starts[e] for e in exps]
    for i, e in enumerate(exps):
        start = neg_starts[e]
        end = neg_sorted[i + 1] if i + 1 < len(exps) else min(pos_starts.values())
        fill(start, end - start, lambda b, n: _ZERO)

    pos_exps = sorted(pos_starts)
    pos_sorted = [pos_starts[e] for e in pos_exps]
    special_base = meta["pos_small_signal_pwl_control"]
    for i, e in enumerate(pos_exps):
        start = pos_starts[e]
        end = pos_sorted[i + 1] if i + 1 < len(pos_exps) else special_base
        n = end - start
        if e <= -1:
            fill(start, n, lambda b, nn: _PIECES[0])
        elif e == 0:
            fill(start, n, lambda b, nn: _PIECES[1])
        elif e == 1:
            def f(b, nn):
                lo = 2.0 + 2.0 * b / nn
                return _PIECES[2] if lo < 3.0 else _PIECES[3]
            fill(start, n, f)
        else:
            fill(start, n, lambda b, nn: _ZERO)

    bkts[meta["pos_small_signal_pwl_control"]] = _PIECES[0]
    bkts[meta["neg_small_signal_pwl_control"]] = _ZERO
    bkts[meta["pos_large_signal_pwl_control"]] = _ZERO
    bkts[meta["neg_large_signal_pwl_control"]] = _ZERO

    for idx, (coeffs, x0) in bkts.items():
        struct.pack_into("<8f", bkt, 32 * idx, *coeffs, x0, 0.0, 0.0, 0.0)
    open(os.path.join(dst, "gelu_and_others_bkt.bin"), "wb").write(bytes(bkt))

    meta["large_pos_signal_exp_threshold"] = 129
    meta["large_pos_signal_mantissa_threshold"] = 0
    meta["large_neg_signal_exp_threshold"] = 129
    meta["large_neg_signal_mantissa_threshold"] = 0
    meta["fpinf_result"] = 0
    meta["fninf_result"] = 0
    meta["fzero_result"] = 0
    json.dump(prof, open(prof_path, "w"))

    h = hashlib.sha256()
    h.update(bytes(bkt))
    h.update(json.dumps(prof, sort_keys=True).encode())
    return os.path.join(dst, "act_info.json"), h.hexdigest()[:16]


# ---------------------------------------------------------------------------
# kernel build
# ---------------------------------------------------------------------------

def _build(n_reps=1, hw_loop=False, rpp=RPP, bufs_t=4, bufs_o=4, mode="full",
           sparse_k=256, act_bypass=False, dual_q=False, layout="p"):
    import concourse.bass as bass
    import concourse.tile as tile
    from concourse import mybir, bacc

    act_info, h = _build_act_root()
    os.environ["BASS_ACT_ROOT_JSON_PATH"] = act_info

    tile_f = rpp * NUM_BASIS
    n_tiles = ROWS_PER_CORE // (128 * rpp)

    nc = bacc.Bacc("TRN2", target_bir_lowering=False, debug=False,
                   num_devices=N_CORES)
    out_name = (f"out_{h}_{n_reps}_{int(hw_loop)}_{rpp}_{bufs_t}{bufs_o}_"
                f"{mode}_{sparse_k}_{int(act_bypass)}_{dual_q}_{layout}")
    RPC = ROWS_PER_CORE // 128          # rows per partition for the core: 2048
    with tile.TileContext(nc) as tc, ExitStack() as ctx:
        x = nc.dram_tensor("x", [ROWS_PER_CORE], mybir.dt.float32,
                           kind="ExternalInput")
        out = nc.dram_tensor(out_name, [ROWS_PER_CORE, NUM_BASIS],
                             mybir.dt.float32, kind="ExternalOutput")
        if layout == "t":
            # tile-major rows: row = t*(128*rpp) + p*rpp + f.  Each tile's
            # output DMA destination is one fully-contiguous 2 MiB block
            # (partition stride 16 KiB == descriptor size).
            xv = x.ap().rearrange("(t p f) -> p t f", p=128, f=rpp)
            ov = out.ap().rearrange("(t p f) j -> t p (f j)", p=128, f=rpp)
        else:
            # partition-major row assignment: row = p*2048 + t*rpp + f
            xv = x.ap().rearrange("(p c) -> p c", p=128)
            ov = out.ap().rearrange("(p t f) j -> t p (f j)", p=128, f=rpp)

        tp = ctx.enter_context(tc.tile_pool(name="tp", bufs=bufs_t))
        op_ = ctx.enter_context(tc.tile_pool(name="op", bufs=bufs_o))

        const_pool = ctx.enter_context(tc.tile_pool(name="const", bufs=1))
        # iota2 constant: value = (j - 33.5), one 64-wide period, broadcast
        # along the row axis at use site.  Temps borrowed from rotating pools.
        iota_i = tp.tile([128, NUM_BASIS], mybir.dt.int32)
        nc.gpsimd.iota(iota_i[:, :NUM_BASIS], pattern=[[1, NUM_BASIS]],
                       base=0, channel_multiplier=0)
        iota_f = op_.tile([128, NUM_BASIS], mybir.dt.float32)
        nc.vector.tensor_copy(iota_f[:, :NUM_BASIS], iota_i[:, :NUM_BASIS])
        iota2 = const_pool.tile([128, NUM_BASIS], mybir.dt.float32)
        nc.vector.tensor_scalar_add(iota2[:, :NUM_BASIS],
                                    iota_f[:, :NUM_BASIS], -INV_H)
        iota2b = iota2[:, :NUM_BASIS].rearrange(
            "p (o j) -> p o j", o=1).broadcast_to([128, rpp, NUM_BASIS])
        # whole x resident in SBUF: one dense 1 MiB DMA (8 KiB/partition)
        xall = const_pool.tile([128, RPC], mybir.dt.float32)
        if layout == "t":
            nc.sync.dma_start(
                xall[:].rearrange("p (t f) -> p t f", f=rpp), xv)
        else:
            nc.sync.dma_start(xall[:], xv)

        if mode == "dma":
            dsrc = const_pool.tile([128, tile_f], mybir.dt.float32)
            nc.vector.memset(dsrc[:], 0.5)

        if mode == "scatter":
            # timing probe: indirect scatter of 32 B windows (8 f32/row),
            # KPI rows per partition per instruction
            KPI = 256
            n_ins = RPC // KPI
            idx = const_pool.tile([128, RPC], mybir.dt.int32)
            nc.gpsimd.iota(idx[:], pattern=[[NUM_BASIS, RPC]], base=0,
                           channel_multiplier=RPC * NUM_BASIS)
            ssrc = const_pool.tile([128, KPI * 8], mybir.dt.float32)
            nc.vector.memset(ssrc[:], 0.25)

        if mode == "sparse":
            K = sparse_k
            n_sp = RPC // K
            # row_iota[p, c] = (p*2048 + c)*64: flat index of (row, col 0)
            row_iota = const_pool.tile([128, RPC], mybir.dt.int32)
            nc.gpsimd.iota(row_iota[:], pattern=[[NUM_BASIS, RPC]], base=0,
                           channel_multiplier=RPC * NUM_BASIS)
            iota8_i = tp.tile([128, 8], mybir.dt.int32)
            nc.gpsimd.iota(iota8_i[:, :8], pattern=[[1, 8]], base=0,
                           channel_multiplier=0)
            iota8f = const_pool.tile([128, 8], mybir.dt.float32)
            nc.vector.tensor_copy(iota8f[:, :8], iota8_i[:, :8])
            iota8b = iota8f[:, :8].rearrange(
                "p (o j) -> p o j", o=1).broadcast_to([128, K, 8])
            sp = ctx.enter_context(tc.tile_pool(name="sp", bufs=bufs_t))

        if mode == "diag":
            # semantics probe: 4 indices/partition, 8 f32 payload each,
            # recognizable values: vals[p, k] = p*100 + k, dest row*64+8
            didx = const_pool.tile([128, 4], mybir.dt.int32)
            nc.gpsimd.iota(didx[:, :4], pattern=[[NUM_BASIS, 4]], base=8,
                           channel_multiplier=RPC * NUM_BASIS)
            dvi = tp.tile([128, 32], mybir.dt.int32)
            nc.gpsimd.iota(dvi[:, :32], pattern=[[1, 32]], base=0,
                           channel_multiplier=100)
            dvf = const_pool.tile([128, 32], mybir.dt.float32)
            nc.vector.tensor_copy(dvf[:, :32], dvi[:, :32])

        if mode == "diag2":
            # per-descriptor offset probe: 4 non-mergeable 8-elem blocks per
            # partition (stride 16), each should consume its own index
            didx = const_pool.tile([128, 4], mybir.dt.int32)
            nc.gpsimd.iota(didx[:, :4], pattern=[[NUM_BASIS, 4]], base=8,
                           channel_multiplier=RPC * NUM_BASIS)
            dvi = tp.tile([128, 64], mybir.dt.int32)
            nc.gpsimd.iota(dvi[:, :64], pattern=[[1, 64]], base=0,
                           channel_multiplier=100)
            dvf = const_pool.tile([128, 64], mybir.dt.float32)
            nc.vector.tensor_copy(dvf[:, :64], dvi[:, :64])

        def one_rep():
            if mode == "diag":
                nc.gpsimd.indirect_dma_start(
                    out.ap(),
                    bass.IndirectOffsetOnAxis(ap=didx[:, :4], axis=1),
                    dvf[:, :32], None)
                return
            if mode == "diag2":
                src = dvf[:, :64].rearrange(
                    "p (c g) -> p c g", g=16)[:, :, :8]
                nc.gpsimd.indirect_dma_start(
                    out.ap(),
                    bass.IndirectOffsetOnAxis(ap=didx[:, :4], axis=1),
                    src, None)
                return
            if mode == "scatter":
                for t in range(n_ins):
                    nc.gpsimd.indirect_dma_start(
                        out.ap(),
                        bass.IndirectOffsetOnAxis(
                            ap=idx[:, t * KPI:(t + 1) * KPI], axis=1),
                        ssrc[:], None)
                return
            if mode == "sparse":
                for t in range(n_sp):
                    xs = xall[:, t * K:(t + 1) * K]
                    # s = (x+1)/h;  sm = s - 3.5 (window-start selector)
                    s = sp.tile([128, K], mybir.dt.float32)
                    nc.vector.tensor_scalar(s[:], xs, INV_H, INV_H,
                                            mybir.AluOpType.mult,
                                            mybir.AluOpType.add)
                    sm = sp.tile([128, K], mybir.dt.float32)
                    nc.vector.tensor_scalar(sm[:], xs, INV_H, INV_H - 3.5,
                                            mybir.AluOpType.mult,
                                            mybir.AluOpType.add)
                    # j0 = clamp(int(s - 3.5), 0, 56): window start column.
                    # correct under either trunc or round-to-nearest convert
                    mi = sp.tile([128, K], mybir.dt.int32)
                    nc.vector.tensor_copy(mi[:], sm[:])
                    j0i = sp.tile([128, K], mybir.dt.int32)
                    nc.vector.tensor_scalar(j0i[:], mi[:], 0, 56,
                                            mybir.AluOpType.max,
                                            mybir.AluOpType.min)
                    # flat element index = row*64 + j0
                    idxt = sp.tile([128, K], mybir.dt.int32)
                    nc.vector.tensor_tensor(idxt[:],
                                            row_iota[:, t * K:(t + 1) * K],
                                            j0i[:], mybir.AluOpType.add)
                    j0f = sp.tile([128, K], mybir.dt.float32)
                    nc.vector.tensor_copy(j0f[:], j0i[:])
                    d = sp.tile([128, K], mybir.dt.float32)
                    nc.vector.tensor_tensor(d[:], s[:], j0f[:],
                                            mybir.AluOpType.subtract)
                    # tt8[p, c, k] = d[p, c] - k;  vals = b3(tt8)
                    db = d[:].rearrange("p (c o) -> p c o", o=1).broadcast_to(
                        [128, K, 8])
                    tt8 = tp.tile([128, K * 8], mybir.dt.float32)
                    nc.vector.scalar_tensor_tensor(
                        tt8[:].rearrange("p (c k) -> p c k", k=8),
                        db, 1.0, iota8b,
                        mybir.AluOpType.mult, mybir.AluOpType.subtract)
                    vals = op_.tile([128, K * 8], mybir.dt.float32)
                    if act_bypass:
                        nc.vector.tensor_copy(vals[:], tt8[:])
                    else:
                        nc.scalar.activation(vals[:], tt8[:],
                                             mybir.ActivationFunctionType.Gelu,
                                             bias=0.0, scale=1.0)
                    nc.gpsimd.indirect_dma_start(
                        out.ap(),
                        bass.IndirectOffsetOnAxis(ap=idxt[:], axis=1),
                        vals[:], None)
                return
            for i in range(n_tiles):
                if mode == "dma":
                    nc.sync.dma_start(ov[i], dsrc[:])
                    continue
                xb = xall[:, i * rpp:(i + 1) * rpp].broadcast_to(
                    [128, rpp, NUM_BASIS])
                tt = tp.tile([128, tile_f], mybir.dt.float32)
                # t = x*33.5 - (j - 33.5) = s - j
                nc.vector.scalar_tensor_tensor(
                    tt[:].rearrange("p (f j) -> p f j", j=NUM_BASIS),
                    xb, INV_H, iota2b,
                    mybir.AluOpType.mult, mybir.AluOpType.subtract)
                ot = op_.tile([128, tile_f], mybir.dt.float32)
                nc.scalar.activation(ot[:], tt[:],
                                     mybir.ActivationFunctionType.Gelu,
                                     bias=0.0, scale=1.0)
                if mode == "full":
                    if dual_q == "split":
                        nc.sync.dma_start(ov[i][0:64, :], ot[0:64, :])
                        nc.scalar.dma_start(ov[i][64:128, :], ot[64:128, :])
                    elif dual_q == "gp4" and (i % 4 == 3):
                        nc.gpsimd.dma_start(ov[i], ot[:])
                    elif dual_q is True and (i % 2):
                        nc.scalar.dma_start(ov[i], ot[:])
                    else:
                        nc.sync.dma_start(ov[i], ot[:])

        if hw_loop and n_reps > 1:
            u = next(c for c in (4, 2, 1) if n_reps % c == 0)
            with tc.For_i(0, n_reps // u):
                for _ in range(u):
                    one_rep()
        else:
            for rep in range(n_reps):
                one_rep()
    nc.compile()
    return nc, out_name


def kernel(x: np.ndarray) -> np.ndarray:
    from concourse import bass_utils

    x = np.ascontiguousarray(x, dtype=np.float32)
    assert x.shape == (N,)
    if "nc" not in _CACHE:
        _CACHE["nc"] = _build()
    nc, out_name = _CACHE["nc"]
    xs = x.reshape(N_CORES, ROWS_PER_CORE)
    in_maps = [{"x": xs[k]} for k in range(N_CORES)]
    res = bass_utils.run_bass_kernel_spmd(nc, in_maps,
                                          core_ids=list(range(N_CORES)))
    out = np.concatenate([r[out_name] for r in res.results], axis=0)
    return out

